# revision 15
# baseline (speedup 1.0000x reference)
"""Causal multi-head self-attention with RoPE on 8 Trainium2 NeuronCores.

Sharding: tensor-parallel over heads — core c owns heads (2c, 2c+1) for BOTH
batch elements.  Feature dim on partitions, tokens on the free dim.

v2: list-scheduled emission.  The PE stream is paced explicitly: the logits
stream (which is throttled by the Activation engine's exp throughput through
the 2-deep logits psum rotation) is interleaved at k-tile granularity with
"filler" matmul quanta — the next chunk's QKV projections, and late in the
kernel the output projections — so the in-order PE sequencer never idles
waiting for exp.  DMAs all ride the SP queue in production order; the three
AllToAlls sit alone on the Pool queue so each dispatches the moment its
staging completes; RoPE multiplies run on GPSIMD (scalar_tensor_tensor) to
unload DVE; softmax reciprocals are batched (one [128,4] strided reciprocal
per head per chunk instead of eight [128,1]s).

  phase A  per 512-token chunk: qT/kT = W @ x^T (f16, K=1024); vT projected
           directly transposed into 130-col k-tiles with a ones column per
           head (denominator comes free out of the AV matmul); RoPE with
           host-precomputed cos/sin (rot = x*C + swap(x)*S, sign folded
           into S).
  phase B  per (batch, q-chunk): logitsT (k-part, q-free) f16 = kT_h^T@qT_h,
           2 heads packed per [128,1024] psum; e = exp(logits/8) -> f16;
           AV transposed: ctx[q-part,65] += e_blk^T @ [v|1]; triangular mask
           on diagonal k-tiles; batched reciprocal + per-q-tile normalize,
           PE-transpose back to (dim, token), stage f16.
  phase C  batch 0: one 8-core AllToAll (512 KB) of half-chunks; batch 1:
           two quarter-chunk AllToAlls (256 KB each).  Local wo^T projection
           per arrival; dummy matmuls keep the PE p-state warm across the
           final collective.  Host reassembles (2, 256, 1024) per core.
"""
import os
import sys
from collections import deque

import numpy as np

for p in ("/opt/trn_rl_repo", "/root/.axon_site/_ro/trn_rl_repo"):
    if os.path.isdir(p) and p not in sys.path:
        sys.path.insert(0, p)

D_MODEL = 1024
NUM_HEADS = 16
D_K = 64
THETA = 10000.0
BATCH = 2
SEQ = 2048
NCORES = 8
H_PER_CORE = 2
DIMS = H_PER_CORE * D_K   # 128 ctx dims owned per core
QC = 512                  # q-chunk
KT = 128                  # k-tile
SCALE = 0.125             # 1/sqrt(d_k)
N_DUMMY = 110

_CACHE = {}


def _build_program():
    import concourse.mybir as mybir
    import concourse.tile as tile
    from concourse import bacc
    from concourse.masks import make_identity, make_upper_triangular

    F32 = mybir.dt.float32
    F16 = mybir.dt.float16
    AFT = mybir.ActivationFunctionType
    ALU = mybir.AluOpType

    nc = bacc.Bacc("TRN2", target_bir_lowering=False, debug=False,
                   num_devices=NCORES)

    xT_d = nc.declare_dram_parameter("xT", [D_MODEL, BATCH * SEQ], F16,
                                     isOutput=False)
    wqT_d = nc.declare_dram_parameter("wqT", [D_MODEL, DIMS], F16, isOutput=False)
    wkT_d = nc.declare_dram_parameter("wkT", [D_MODEL, DIMS], F16, isOutput=False)
    wvT_d = nc.declare_dram_parameter("wvT", [D_MODEL, DIMS], F16, isOutput=False)
    woT_d = nc.declare_dram_parameter("woT", [D_MODEL, D_MODEL], F16, isOutput=False)
    csC_d = nc.declare_dram_parameter("csC", [DIMS, BATCH * SEQ], F16,
                                      isOutput=False)
    csS_d = nc.declare_dram_parameter("csS", [DIMS, BATCH * SEQ], F16,
                                      isOutput=False)
    psw_d = nc.declare_dram_parameter("psw", [128, 128], F16, isOutput=False)
    out_d = nc.declare_dram_parameter("out", [BATCH, 2 * KT, D_MODEL], F16,
                                      isOutput=True)

    NCH = SEQ // QC           # 4 chunks per batch
    NVT = QC // KT            # 4 k-tiles per chunk

    with tile.TileContext(nc) as tc:
        with tc.tile_pool(name="consts", bufs=1) as consts, \
             tc.tile_pool(name="qk", bufs=1) as qkp, \
             tc.tile_pool(name="vbufp", bufs=1) as vbufp, \
             tc.tile_pool(name="ps", bufs=1, space="PSUM") as ps, \
             tc.tile_pool(name="epool", bufs=26) as epool, \
             tc.tile_pool(name="rawp", bufs=2) as rawp, \
             tc.tile_pool(name="xtp", bufs=1) as xtp, \
             tc.tile_pool(name="ropep", bufs=2) as ropep, \
             tc.tile_pool(name="normp", bufs=3) as normp, \
             tc.tile_pool(name="stp", bufs=2) as stp, \
             tc.tile_pool(name="wp", bufs=1) as wp, \
             tc.tile_pool(name="outp", bufs=2) as outp, \
             tc.tile_pool(name="dram", bufs=1, space="DRAM") as dram:

            # ---------- constants ----------
            tri_f = consts.tile([KT, KT], F32)
            make_upper_triangular(nc, tri_f[:], val=1.0, diag=True)
            tri = consts.tile([KT, KT], F16)
            nc.vector.tensor_copy(tri, tri_f)
            ident = consts.tile([128, 128], F32)
            make_identity(nc, ident[:])
            ones16 = consts.tile([128, 16], F16)
            nc.vector.memset(ones16, 1.0)

            csC = consts.tile([DIMS, BATCH * SEQ], F16, name="csC")
            csS = consts.tile([DIMS, BATCH * SEQ], F16, name="csS")
            psw = consts.tile([128, 128], F16, name="psw")

            a2a_in = [dram.tile([NCORES, DIMS, 2 * KT], F16, name=f"a2ain{b}")
                      for b in range(BATCH)]
            a2a_out = [dram.tile([NCORES, DIMS, 2 * KT], F16, name=f"a2aout{b}")
                       for b in range(BATCH)]

            qR = {b: qkp.tile([DIMS, SEQ], F16, tag=f"qR{b}", name=f"qR{b}")
                  for b in range(BATCH)}
            kR = {b: qkp.tile([DIMS, SEQ], F16, tag=f"kR{b}", name=f"kR{b}")
                  for b in range(BATCH)}
            vbuf = {b: vbufp.tile([128, 130 * (SEQ // KT)], F16, tag=f"vb{b}",
                                  name=f"vbuf{b}")
                    for b in range(BATCH)}

            # projection weights (wq first so the very first matmul can start
            # as soon as wq + the first x chunk land)
            w_sb = {}
            for nm, d in (("q", wqT_d), ("k", wkT_d), ("v", wvT_d)):
                wt = wp.tile([128, 8, DIMS], F16, tag=f"w{nm}", name=f"w{nm}")
                w_sb[nm] = [wt[:, k8, :] for k8 in range(8)]
                w_sb[nm + "_t"] = wt

            xt_all = [xtp.tile([128, BATCH * SEQ], F16, tag=f"xt{k8}",
                               name=f"xt{k8}")
                      for k8 in range(8)]

            def load_x(c0, c1):
                for k8 in range(8):
                    nc.sync.dma_start(
                        out=xt_all[k8][:, c0:c1],
                        in_=xT_d[k8 * 128:(k8 + 1) * 128, c0:c1])

            def load_w(nm, d):
                nc.sync.dma_start(
                    out=w_sb[nm + "_t"],
                    in_=d[:].rearrange("(e p) c -> p e c", p=128))

            def load_cs(c0, c1):
                nc.sync.dma_start(out=csC[:, c0:c1], in_=csC_d[:, c0:c1])
                nc.sync.dma_start(out=csS[:, c0:c1], in_=csS_d[:, c0:c1])

            load_w("q", wqT_d)
            for k8 in range(4):
                nc.sync.dma_start(out=xt_all[k8][:, 0:QC],
                                  in_=xT_d[k8 * 128:(k8 + 1) * 128, 0:QC])
            load_w("k", wkT_d)
            for k8 in range(4, 8):
                nc.sync.dma_start(out=xt_all[k8][:, 0:QC],
                                  in_=xT_d[k8 * 128:(k8 + 1) * 128, 0:QC])
            load_cs(0, QC)
            load_w("v", wvT_d)
            nc.sync.dma_start(out=psw, in_=psw_d[:])
            load_x(QC, SEQ)        # rest of batch 0, 8 big DMAs
            load_cs(QC, SEQ)
            load_cs(SEQ, 2 * SEQ)
            load_x(SEQ, 2 * SEQ)   # batch 1, 8 big DMAs

            def emit_wo_loads():
                t = wp.tile([128, 8, D_MODEL], F16, tag="wo", name="wo")
                nc.sync.dma_start(
                    out=t, in_=woT_d[:].rearrange("(e p) c -> p e c", p=128))
                return [t[:, k8, :] for k8 in range(8)]

            # ---------- phase A: one 512-token chunk, as filler quanta ----
            # generator yields (approx_pe_ns) after each quantum
            def gen_chunk(b, t):
                g0 = b * SEQ + t * QC
                c0 = t * QC
                rqk = rawp.tile([DIMS, 2 * QC], F16, tag="rawqk", name="rawqk")
                for ni, nm in enumerate(("q", "k")):
                    pp = ps.tile([128, QC], F32, tag="m1", bufs=2, name="pp")
                    for k8 in range(8):
                        nc.tensor.matmul(pp, w_sb[nm][k8],
                                         xt_all[k8][:, g0:g0 + QC],
                                         start=(k8 == 0), stop=(k8 == 7))
                        if k8 % 2 == 1:
                            yield 426
                    nc.vector.tensor_copy(rqk[:, ni * QC:(ni + 1) * QC], pp)

                # v: projected directly transposed, (token, dim) per 128-tile
                pv = ps.tile([128, QC], F32, tag="m1", bufs=2, name="pv")
                for i in range(NVT):
                    for k8 in range(8):
                        nc.tensor.matmul(
                            pv[:, i * KT:(i + 1) * KT],
                            xt_all[k8][:, g0 + i * KT:g0 + (i + 1) * KT],
                            w_sb["v"][k8],
                            start=(k8 == 0), stop=(k8 == 7),
                            skip_group_check=True)
                    yield 426
                vb = vbuf[b]
                cview = vb[:, 130 * NVT * t:130 * NVT * (t + 1)]
                v_view = cview.rearrange("p (n c) -> p n c", c=130)
                for col in (64, 129):
                    nc.vector.tensor_copy(
                        v_view[:, :, col:col + 1].rearrange("p n c -> p (n c)"),
                        ones16[:, 0:NVT])
                dst = cview.rearrange("p (n h c) -> p n h c", h=2, c=65)[
                    :, :, :, 0:64]
                src = pv[:].rearrange("p (n h c) -> p n h c", h=2, c=64)
                nc.vector.tensor_copy(dst, src)

                # RoPE: rot = x*C + swap(x)*S (sign in S).  The even/odd
                # row swap is a PE permutation matmul into psum; the S-mul
                # reads the psum directly.
                for ni, nm in enumerate(("q", "k")):
                    nsl = slice(ni * QC, (ni + 1) * QC)
                    sw = ps.tile([128, QC], F32, tag="m1", bufs=2, name="sw")
                    nc.tensor.matmul(sw, psw, rqk[:, nsl],
                                     start=True, stop=True)
                    yield 213
                    t1 = ropep.tile([DIMS, QC], F16, tag=f"t1{ni}",
                                    name="t1")
                    swm = ropep.tile([DIMS, QC], F16, tag=f"sw{ni}",
                                     name="swm")
                    nc.vector.tensor_mul(t1, rqk[:, nsl], csC[:, g0:g0 + QC])
                    nc.vector.tensor_mul(swm, sw, csS[:, g0:g0 + QC])
                    dst = (qR if nm == "q" else kR)[b][:, c0:c0 + QC]
                    nc.vector.tensor_add(dst, t1, swm)
                yield 0

            def gen_dummy(n):
                for _ in range(n):
                    dpl = ps.tile([128, QC], F32, tag="m1", bufs=2,
                                  name="dpl")
                    nc.tensor.matmul(dpl[0:16, 0:256], ones16,
                                     xt_all[0][:, 0:256],
                                     start=True, stop=True)
                    yield 107

            # ---------- filler management ----------
            filler = deque()   # (pe_ns, generator)

            def push_filler(gen):
                filler.append(gen)

            def pop_filler(budget_ns):
                spent = 0
                while filler and spent < budget_ns:
                    g = filler[0]
                    try:
                        spent += next(g)
                    except StopIteration:
                        filler.popleft()
                return spent

            def flush_filler():
                while filler:
                    g = filler[0]
                    try:
                        next(g)
                    except StopIteration:
                        filler.popleft()

            # ---------- phase B ----------
            ets_store = {}

            def gen_logits(b, qc):
                nkt = NVT * qc + NVT
                ets = []
                ets_store[(b, qc)] = ets
                for kt in range(nkt):
                    j = kt - NVT * qc
                    q0 = 0 if j < 0 else KT * j
                    pl = ps.tile([128, 2 * QC], F32, tag="logit", bufs=2,
                                 name="pl")
                    for h in range(H_PER_CORE):
                        nc.tensor.matmul(
                            pl[:, h * QC + q0:(h + 1) * QC],
                            kR[b][64 * h:64 * (h + 1), kt * KT:(kt + 1) * KT],
                            qR[b][64 * h:64 * (h + 1),
                                  qc * QC + q0:(qc + 1) * QC],
                            start=True, stop=True)
                    et = epool.tile([128, 2 * QC], F16, tag="e", name="et")
                    if q0 == 0:
                        nc.scalar.activation(et, pl, AFT.Exp, scale=SCALE)
                    else:
                        ev = et[:].rearrange("p (h n) -> p h n", h=2)[
                            :, :, q0:QC]
                        pv_ = pl[:].rearrange("p (h n) -> p h n", h=2)[
                            :, :, q0:QC]
                        nc.scalar.activation(ev, pv_, AFT.Exp, scale=SCALE)
                    if j >= 0:
                        for h in range(H_PER_CORE):
                            msl = slice(h * QC + q0, h * QC + q0 + KT)
                            nc.vector.tensor_mul(et[:, msl], et[:, msl], tri)
                    ets.append(et)
                    # exp takes ~975ns/kt on Act; the 2 logits mms are
                    # ~426ns: pad the difference with filler
                    pop_filler(550)
                    yield

            def gen_av(b, qc):
                ets = ets_store.pop((b, qc))
                pctx = [ps.tile([128, QC], F32, tag=f"ctx{h}", bufs=1,
                                name=f"pctx{h}")
                        for h in range(H_PER_CORE)]
                for qt in range(NVT):
                    cnt = 0
                    for kt in range(NVT * qc + qt + 1):
                        for h in range(H_PER_CORE):
                            vt = vbuf[b][:, 130 * kt + 65 * h:
                                         130 * kt + 65 * h + 65]
                            nc.tensor.matmul(
                                pctx[h][:, qt * KT:qt * KT + 65],
                                ets[kt][:, h * QC + qt * KT:
                                        h * QC + (qt + 1) * KT],
                                vt,
                                start=(kt == 0),
                                stop=(kt == NVT * qc + qt),
                                skip_group_check=True)
                        cnt += 1
                        if cnt % 6 == 0:
                            yield 324
                    yield 110

                # epilogue: batched reciprocal, normalize, transpose, stage
                rc = {}
                for h in range(H_PER_CORE):
                    rc[h] = normp.tile([128, NVT], F32, tag=f"rc{h}",
                                       name="rc")
                    den = pctx[h][:].rearrange("p (n c) -> p n c", c=KT)[
                        :, :, 64:65].rearrange("p n c -> p (n c)")
                    nc.vector.reciprocal(rc[h], den)
                stg = stp.tile([128, QC], F16, tag="stage", name="stage")
                for qt in range(NVT):
                    tp = ps.tile([128, QC], F32, tag="m1", bufs=2, name="tp")
                    nr = normp.tile([128, 128], F32, tag="nr", name="nr")
                    for h in range(H_PER_CORE):
                        nc.vector.tensor_scalar_mul(
                            nr[:, 64 * h:64 * (h + 1)],
                            pctx[h][:, qt * KT:qt * KT + 64],
                            rc[h][:, qt:qt + 1])
                    nc.tensor.transpose(tp[:, 0:128], nr, ident[:])
                    nc.vector.tensor_copy(stg[:, qt * KT:(qt + 1) * KT],
                                          tp[:, 0:128])
                nc.sync.dma_start(
                    out=a2a_in[b][2 * qc:2 * qc + 2].rearrange(
                        "e p c -> p e c"),
                    in_=stg[:].rearrange("p (e c) -> p e c", e=2))

            def emit_a2a(b):
                nc.gpsimd.collective_compute(
                    "AllToAll", mybir.AluOpType.bypass,
                    replica_groups=[list(range(NCORES))],
                    ins=[a2a_in[b].opt()], outs=[a2a_out[b].opt()],
                )

            # ---------- phase C: output projections (as quanta) ----------
            def gen_woproj(b, wo_sb, cm):
                for mt in range(2):
                    ot = outp.tile([128, D_MODEL], F16, tag="out", name="ot")
                    for nn in range(2):
                        po = ps.tile([128, QC], F32, tag="m1", bufs=2,
                                     name="po")
                        for i in range(NCORES):
                            nc.tensor.matmul(
                                po, cm[:, i, mt * KT:(mt + 1) * KT],
                                wo_sb[i][:, nn * QC:(nn + 1) * QC],
                                start=(i == 0), stop=(i == NCORES - 1))
                            if i % 2 == 1:
                                yield 426
                        nc.vector.tensor_copy(ot[:, nn * QC:(nn + 1) * QC], po)
                    nc.sync.dma_start(
                        out=out_d[b, mt * KT:(mt + 1) * KT, :],
                        in_=ot[:])
                yield 0

            # ---------- emission schedule ----------
            for _ in gen_chunk(0, 0):
                pass
            wo_sb = emit_wo_loads()

            blocks = [(b, qc) for b in range(BATCH) for qc in range(NCH)]
            next_chunk = {0: (0, 1), 1: (0, 2), 2: (0, 3), 3: (1, 0),
                          4: (1, 1), 5: (1, 2), 6: (1, 3)}
            prev = None
            dummy_gen = None
            for i, blk in enumerate(blocks):
                if i in next_chunk:
                    push_filler(gen_chunk(*next_chunk[i]))
                if i == 7:
                    dummy_gen = gen_dummy(N_DUMMY)
                    push_filler(dummy_gen)
                g = gen_logits(*blk)
                for _ in g:
                    pass
                if dummy_gen is not None and filler and filler[-1] is dummy_gen:
                    filler.pop()           # keep dummies out of the flush
                flush_filler()
                if prev == (0, 3):
                    emit_a2a(0)
                if dummy_gen is not None:
                    filler.append(dummy_gen)
                prev = blk
                if i < 7:
                    push_filler(gen_av(*blk))
                else:
                    if filler and filler[-1] is dummy_gen:
                        filler.pop()
                    flush_filler()         # any av leftovers
                    for _ in gen_av(*blk):
                        pass
            emit_a2a(1)
            cm0 = wp.tile([DIMS, NCORES, 2 * KT], F16, tag="cm0", name="cm0")
            nc.sync.dma_start(
                out=cm0, in_=a2a_out[0][:].rearrange("e p c -> p e c"))
            cm1 = wp.tile([DIMS, NCORES, 2 * KT], F16, tag="cm1", name="cm1")
            nc.sync.dma_start(
                out=cm1, in_=a2a_out[1][:].rearrange("e p c -> p e c"))

            for _ in gen_woproj(0, wo_sb, cm0):
                pass
            # keep PE p-state warm while the last AllToAll is in flight
            if dummy_gen is not None:
                for _ in dummy_gen:
                    pass
            for _ in gen_woproj(1, wo_sb, cm1):
                pass

    nc.compile()
    return nc


def _host_prep(inputs):
    x = np.asarray(inputs["in_features"], dtype=np.float32)
    tp = np.asarray(inputs["token_positions"], dtype=np.int32)
    wq = np.asarray(inputs["wq"], dtype=np.float32)
    wk = np.asarray(inputs["wk"], dtype=np.float32)
    wv = np.asarray(inputs["wv"], dtype=np.float32)
    wo = np.asarray(inputs["wo"], dtype=np.float32)

    xT = np.ascontiguousarray(
        np.concatenate([x[b].T for b in range(BATCH)], axis=1)).astype(np.float16)
    woT = np.ascontiguousarray(wo.T).astype(np.float16)

    # cos/sin tables, (dim row, batch*token col); sign baked into S so that
    # rot = x*C + swap(x)*S
    half = D_K // 2
    inv_freq = 1.0 / (THETA ** (2.0 * np.arange(half) / D_K))     # (32,)
    ang = tp.astype(np.float64)[:, :, None] * inv_freq[None, None, :]
    cos = np.cos(ang)                                             # (B, S, 32)
    sin = np.sin(ang)
    rows = np.arange(DIMS)
    j = (rows % D_K) // 2                                         # freq index
    sign = np.where(rows % 2 == 0, -1.0, 1.0)
    csC = np.empty((DIMS, BATCH * SEQ), dtype=np.float16)
    csS = np.empty((DIMS, BATCH * SEQ), dtype=np.float16)
    for b in range(BATCH):
        csC[:, b * SEQ:(b + 1) * SEQ] = cos[b][:, j].T
        csS[:, b * SEQ:(b + 1) * SEQ] = (sin[b][:, j] * sign[None, :]).T

    psw = np.zeros((128, 128), dtype=np.float16)
    r = np.arange(128)
    psw[r ^ 1, r] = 1.0

    in_maps = []
    for c in range(NCORES):
        rsl = slice(DIMS * c, DIMS * (c + 1))
        in_maps.append({
            "xT": xT,
            "psw": psw,
            "wqT": np.ascontiguousarray(wq[rsl].T).astype(np.float16),
            "wkT": np.ascontiguousarray(wk[rsl].T).astype(np.float16),
            "wvT": np.ascontiguousarray(wv[rsl].T).astype(np.float16),
            "woT": woT,
            "csC": csC,
            "csS": csS,
        })
    return in_maps


def kernel(**inputs) -> np.ndarray:
    from concourse.bass_utils import run_bass_kernel_spmd

    if "nc" not in _CACHE:
        _CACHE["nc"] = _build_program()
    nc = _CACHE["nc"]

    in_maps = _host_prep(inputs)
    res = run_bass_kernel_spmd(nc, in_maps, list(range(NCORES))).results

    out = np.empty((BATCH, SEQ, D_MODEL), dtype=np.float32)
    for c in range(NCORES):
        # half-chunk layout for both batches (qc = c//2, half = c%2)
        t0 = (c // 2) * QC + (c % 2) * 2 * KT
        for b in range(BATCH):
            out[b, t0:t0 + 2 * KT, :] = res[c]["out"][b].astype(np.float32)
    return out


# revision 16
# speedup vs baseline: 1.0369x; 1.0369x over previous
"""Causal multi-head self-attention with RoPE on 8 Trainium2 NeuronCores.

Sharding: tensor-parallel over heads — core c owns heads (2c, 2c+1) for BOTH
batch elements.  Feature dim on partitions, tokens on the free dim.

v2: list-scheduled emission.  The PE stream is paced explicitly: the logits
stream (which is throttled by the Activation engine's exp throughput through
the 2-deep logits psum rotation) is interleaved at k-tile granularity with
"filler" matmul quanta — the next chunk's QKV projections, and late in the
kernel the output projections — so the in-order PE sequencer never idles
waiting for exp.  DMAs all ride the SP queue in production order; the three
AllToAlls sit alone on the Pool queue so each dispatches the moment its
staging completes; RoPE multiplies run on GPSIMD (scalar_tensor_tensor) to
unload DVE; softmax reciprocals are batched (one [128,4] strided reciprocal
per head per chunk instead of eight [128,1]s).

  phase A  per 512-token chunk: qT/kT = W @ x^T (f16, K=1024); vT projected
           directly transposed into 130-col k-tiles with a ones column per
           head (denominator comes free out of the AV matmul); RoPE with
           host-precomputed cos/sin (rot = x*C + swap(x)*S, sign folded
           into S).
  phase B  per (batch, q-chunk): logitsT (k-part, q-free) f16 = kT_h^T@qT_h,
           2 heads packed per [128,1024] psum; e = exp(logits/8) -> f16;
           AV transposed: ctx[q-part,65] += e_blk^T @ [v|1]; triangular mask
           on diagonal k-tiles; batched reciprocal + per-q-tile normalize,
           PE-transpose back to (dim, token), stage f16.
  phase C  batch 0: one 8-core AllToAll (512 KB) of half-chunks; batch 1:
           two quarter-chunk AllToAlls (256 KB each).  Local wo^T projection
           per arrival; dummy matmuls keep the PE p-state warm across the
           final collective.  Host reassembles (2, 256, 1024) per core.
"""
import os
import sys
from collections import deque

import numpy as np

for p in ("/opt/trn_rl_repo", "/root/.axon_site/_ro/trn_rl_repo"):
    if os.path.isdir(p) and p not in sys.path:
        sys.path.insert(0, p)

D_MODEL = 1024
NUM_HEADS = 16
D_K = 64
THETA = 10000.0
BATCH = 2
SEQ = 2048
NCORES = 8
H_PER_CORE = 2
DIMS = H_PER_CORE * D_K   # 128 ctx dims owned per core
QC = 512                  # q-chunk
KT = 128                  # k-tile
SCALE = 0.125             # 1/sqrt(d_k)
N_DUMMY = 260

_CACHE = {}


def _build_program():
    import concourse.mybir as mybir
    import concourse.tile as tile
    from concourse import bacc
    from concourse.masks import make_identity, make_upper_triangular

    F32 = mybir.dt.float32
    F16 = mybir.dt.float16
    AFT = mybir.ActivationFunctionType
    ALU = mybir.AluOpType

    nc = bacc.Bacc("TRN2", target_bir_lowering=False, debug=False,
                   num_devices=NCORES)

    xT_d = nc.declare_dram_parameter("xT", [D_MODEL, BATCH * SEQ], F16,
                                     isOutput=False)
    wqT_d = nc.declare_dram_parameter("wqT", [D_MODEL, DIMS], F16, isOutput=False)
    wkT_d = nc.declare_dram_parameter("wkT", [D_MODEL, DIMS], F16, isOutput=False)
    wvT_d = nc.declare_dram_parameter("wvT", [D_MODEL, DIMS], F16, isOutput=False)
    woT_d = nc.declare_dram_parameter("woT", [D_MODEL, D_MODEL], F16, isOutput=False)
    csC_d = nc.declare_dram_parameter("csC", [DIMS, BATCH * SEQ], F16,
                                      isOutput=False)
    csS_d = nc.declare_dram_parameter("csS", [DIMS, BATCH * SEQ], F16,
                                      isOutput=False)
    psw_d = nc.declare_dram_parameter("psw", [128, 128], F16, isOutput=False)
    out_d = nc.declare_dram_parameter("out", [BATCH, 2 * KT, D_MODEL], F16,
                                      isOutput=True)

    NCH = SEQ // QC           # 4 chunks per batch
    NVT = QC // KT            # 4 k-tiles per chunk

    with tile.TileContext(nc) as tc:
        with tc.tile_pool(name="consts", bufs=1) as consts, \
             tc.tile_pool(name="qk", bufs=1) as qkp, \
             tc.tile_pool(name="vbufp", bufs=1) as vbufp, \
             tc.tile_pool(name="ps", bufs=1, space="PSUM") as ps, \
             tc.tile_pool(name="epool", bufs=26) as epool, \
             tc.tile_pool(name="rawp", bufs=2) as rawp, \
             tc.tile_pool(name="xtp", bufs=1) as xtp, \
             tc.tile_pool(name="ropep", bufs=2) as ropep, \
             tc.tile_pool(name="normp", bufs=3) as normp, \
             tc.tile_pool(name="stp", bufs=2) as stp, \
             tc.tile_pool(name="wp", bufs=1) as wp, \
             tc.tile_pool(name="outp", bufs=2) as outp, \
             tc.tile_pool(name="dram", bufs=1, space="DRAM") as dram:

            # ---------- constants ----------
            tri_f = consts.tile([KT, KT], F32)
            make_upper_triangular(nc, tri_f[:], val=1.0, diag=True)
            tri = consts.tile([KT, KT], F16)
            nc.vector.tensor_copy(tri, tri_f)
            ident = consts.tile([128, 128], F32)
            make_identity(nc, ident[:])
            ones16 = consts.tile([128, 16], F16)
            nc.vector.memset(ones16, 1.0)

            csC = consts.tile([DIMS, BATCH * SEQ], F16, name="csC")
            csS = consts.tile([DIMS, BATCH * SEQ], F16, name="csS")
            psw = consts.tile([128, 128], F16, name="psw")

            a2a_in = [dram.tile([NCORES, DIMS, 2 * KT], F16, name=f"a2ain{b}")
                      for b in range(BATCH)]
            a2a_out = [dram.tile([NCORES, DIMS, 2 * KT], F16, name=f"a2aout{b}")
                       for b in range(BATCH)]

            qR = {b: qkp.tile([DIMS, SEQ], F16, tag=f"qR{b}", name=f"qR{b}")
                  for b in range(BATCH)}
            kR = {b: qkp.tile([DIMS, SEQ], F16, tag=f"kR{b}", name=f"kR{b}")
                  for b in range(BATCH)}
            vbuf = {b: vbufp.tile([128, 130 * (SEQ // KT)], F16, tag=f"vb{b}",
                                  name=f"vbuf{b}")
                    for b in range(BATCH)}

            # projection weights (wq first so the very first matmul can start
            # as soon as wq + the first x chunk land)
            w_sb = {}
            for nm, d in (("q", wqT_d), ("k", wkT_d), ("v", wvT_d)):
                wt = wp.tile([128, 8, DIMS], F16, tag=f"w{nm}", name=f"w{nm}")
                w_sb[nm] = [wt[:, k8, :] for k8 in range(8)]
                w_sb[nm + "_t"] = wt

            xt_all = [xtp.tile([128, BATCH * SEQ], F16, tag=f"xt{k8}",
                               name=f"xt{k8}")
                      for k8 in range(8)]

            def load_x(c0, c1):
                for k8 in range(8):
                    nc.sync.dma_start(
                        out=xt_all[k8][:, c0:c1],
                        in_=xT_d[k8 * 128:(k8 + 1) * 128, c0:c1])

            def load_w(nm, d):
                nc.sync.dma_start(
                    out=w_sb[nm + "_t"],
                    in_=d[:].rearrange("(e p) c -> p e c", p=128))

            def load_cs(c0, c1):
                nc.sync.dma_start(out=csC[:, c0:c1], in_=csC_d[:, c0:c1])
                nc.sync.dma_start(out=csS[:, c0:c1], in_=csS_d[:, c0:c1])

            load_w("q", wqT_d)
            for k8 in range(4):
                nc.sync.dma_start(out=xt_all[k8][:, 0:QC],
                                  in_=xT_d[k8 * 128:(k8 + 1) * 128, 0:QC])
            load_w("k", wkT_d)
            for k8 in range(4, 8):
                nc.sync.dma_start(out=xt_all[k8][:, 0:QC],
                                  in_=xT_d[k8 * 128:(k8 + 1) * 128, 0:QC])
            load_cs(0, QC)
            load_w("v", wvT_d)
            nc.sync.dma_start(out=psw, in_=psw_d[:])
            load_x(QC, SEQ)        # rest of batch 0, 8 big DMAs
            load_cs(QC, SEQ)
            load_cs(SEQ, 2 * SEQ)
            load_x(SEQ, 2 * SEQ)   # batch 1, 8 big DMAs

            def emit_wo_loads():
                t = wp.tile([128, 8, D_MODEL], F16, tag="wo", name="wo")
                nc.sync.dma_start(
                    out=t, in_=woT_d[:].rearrange("(e p) c -> p e c", p=128))
                return [t[:, k8, :] for k8 in range(8)]

            # ---------- phase A: one 512-token chunk, as filler quanta ----
            # generator yields (approx_pe_ns) after each quantum
            def gen_chunk(b, t):
                g0 = b * SEQ + t * QC
                c0 = t * QC
                rqk = rawp.tile([DIMS, 2 * QC], F16, tag="rawqk", name="rawqk")
                for ni, nm in enumerate(("q", "k")):
                    pp = ps.tile([128, QC], F32, tag="m1", bufs=2, name="pp")
                    for k8 in range(8):
                        nc.tensor.matmul(pp, w_sb[nm][k8],
                                         xt_all[k8][:, g0:g0 + QC],
                                         start=(k8 == 0), stop=(k8 == 7))
                        if k8 % 2 == 1:
                            yield 426
                    nc.vector.tensor_copy(rqk[:, ni * QC:(ni + 1) * QC], pp)

                # v: projected directly transposed, (token, dim) per 128-tile
                pv = ps.tile([128, QC], F32, tag="m1", bufs=2, name="pv")
                for i in range(NVT):
                    for k8 in range(8):
                        nc.tensor.matmul(
                            pv[:, i * KT:(i + 1) * KT],
                            xt_all[k8][:, g0 + i * KT:g0 + (i + 1) * KT],
                            w_sb["v"][k8],
                            start=(k8 == 0), stop=(k8 == 7),
                            skip_group_check=True)
                    yield 426
                vb = vbuf[b]
                cview = vb[:, 130 * NVT * t:130 * NVT * (t + 1)]
                v_view = cview.rearrange("p (n c) -> p n c", c=130)
                for col in (64, 129):
                    nc.vector.tensor_copy(
                        v_view[:, :, col:col + 1].rearrange("p n c -> p (n c)"),
                        ones16[:, 0:NVT])
                dst = cview.rearrange("p (n h c) -> p n h c", h=2, c=65)[
                    :, :, :, 0:64]
                src = pv[:].rearrange("p (n h c) -> p n h c", h=2, c=64)
                nc.vector.tensor_copy(dst, src)

                # RoPE: rot = x*C + swap(x)*S (sign in S).  The even/odd
                # row swap is a PE permutation matmul into psum; the S-mul
                # reads the psum directly.
                for ni, nm in enumerate(("q", "k")):
                    nsl = slice(ni * QC, (ni + 1) * QC)
                    sw = ps.tile([128, QC], F32, tag="m1", bufs=2, name="sw")
                    nc.tensor.matmul(sw, psw, rqk[:, nsl],
                                     start=True, stop=True)
                    yield 213
                    t1 = ropep.tile([DIMS, QC], F16, tag=f"t1{ni}",
                                    name="t1")
                    swm = ropep.tile([DIMS, QC], F16, tag=f"sw{ni}",
                                     name="swm")
                    nc.vector.tensor_mul(t1, rqk[:, nsl], csC[:, g0:g0 + QC])
                    nc.vector.tensor_mul(swm, sw, csS[:, g0:g0 + QC])
                    dst = (qR if nm == "q" else kR)[b][:, c0:c0 + QC]
                    nc.vector.tensor_add(dst, t1, swm)
                yield 0

            def gen_dummy(n):
                for _ in range(n):
                    dpl = ps.tile([128, QC], F32, tag="m1", bufs=2,
                                  name="dpl")
                    nc.tensor.matmul(dpl[0:16, 0:256], ones16,
                                     xt_all[0][:, 0:256],
                                     start=True, stop=True)
                    yield 107

            # ---------- filler management ----------
            filler = deque()   # (pe_ns, generator)

            def push_filler(gen):
                filler.append(gen)

            def pop_filler(budget_ns):
                spent = 0
                while filler and spent < budget_ns:
                    g = filler[0]
                    try:
                        spent += next(g)
                    except StopIteration:
                        filler.popleft()
                return spent

            def flush_filler():
                while filler:
                    g = filler[0]
                    try:
                        next(g)
                    except StopIteration:
                        filler.popleft()

            def flush_until_done(cg):
                while any(g is cg for g in filler):
                    g = filler[0]
                    try:
                        next(g)
                    except StopIteration:
                        filler.popleft()

            # ---------- phase B ----------
            ets_store = {}

            def gen_logits(b, qc):
                nkt = NVT * qc + NVT
                ets = []
                ets_store[(b, qc)] = ets
                for kt in range(nkt):
                    j = kt - NVT * qc
                    q0 = 0 if j < 0 else KT * j
                    pl = ps.tile([128, 2 * QC], F32, tag="logit", bufs=2,
                                 name="pl")
                    for h in range(H_PER_CORE):
                        nc.tensor.matmul(
                            pl[:, h * QC + q0:(h + 1) * QC],
                            kR[b][64 * h:64 * (h + 1), kt * KT:(kt + 1) * KT],
                            qR[b][64 * h:64 * (h + 1),
                                  qc * QC + q0:(qc + 1) * QC],
                            start=True, stop=True)
                    et = epool.tile([128, 2 * QC], F16, tag="e", name="et")
                    if q0 == 0:
                        nc.scalar.activation(et, pl, AFT.Exp, scale=SCALE)
                    else:
                        ev = et[:].rearrange("p (h n) -> p h n", h=2)[
                            :, :, q0:QC]
                        pv_ = pl[:].rearrange("p (h n) -> p h n", h=2)[
                            :, :, q0:QC]
                        nc.scalar.activation(ev, pv_, AFT.Exp, scale=SCALE)
                    if j >= 0:
                        for h in range(H_PER_CORE):
                            msl = slice(h * QC + q0, h * QC + q0 + KT)
                            nc.vector.tensor_mul(et[:, msl], et[:, msl], tri)
                    ets.append(et)
                    # exp takes ~975ns/kt on Act; the 2 logits mms are
                    # ~426ns: pad the difference with filler
                    pop_filler(550)
                    yield

            def gen_av(b, qc):
                ets = ets_store.pop((b, qc))
                pctx = [ps.tile([128, QC], F32, tag=f"ctx{h}", bufs=1,
                                name=f"pctx{h}")
                        for h in range(H_PER_CORE)]
                for qt in range(NVT):
                    cnt = 0
                    for kt in range(NVT * qc + qt + 1):
                        for h in range(H_PER_CORE):
                            vt = vbuf[b][:, 130 * kt + 65 * h:
                                         130 * kt + 65 * h + 65]
                            nc.tensor.matmul(
                                pctx[h][:, qt * KT:qt * KT + 65],
                                ets[kt][:, h * QC + qt * KT:
                                        h * QC + (qt + 1) * KT],
                                vt,
                                start=(kt == 0),
                                stop=(kt == NVT * qc + qt),
                                skip_group_check=True)
                        cnt += 1
                        if cnt % 6 == 0:
                            yield 324
                    yield 110

                # epilogue: batched reciprocal, normalize, transpose, stage
                rc = {}
                for h in range(H_PER_CORE):
                    rc[h] = normp.tile([128, NVT], F32, tag=f"rc{h}",
                                       name="rc")
                    den = pctx[h][:].rearrange("p (n c) -> p n c", c=KT)[
                        :, :, 64:65].rearrange("p n c -> p (n c)")
                    nc.vector.reciprocal(rc[h], den)
                stg = stp.tile([128, QC], F16, tag="stage", name="stage")
                for qt in range(NVT):
                    tp = ps.tile([128, QC], F32, tag="m1", bufs=2, name="tp")
                    nr = normp.tile([128, 128], F32, tag="nr", name="nr")
                    for h in range(H_PER_CORE):
                        nc.vector.tensor_scalar_mul(
                            nr[:, 64 * h:64 * (h + 1)],
                            pctx[h][:, qt * KT:qt * KT + 64],
                            rc[h][:, qt:qt + 1])
                    nc.tensor.transpose(tp[:, 0:128], nr, ident[:])
                    nc.vector.tensor_copy(stg[:, qt * KT:(qt + 1) * KT],
                                          tp[:, 0:128])
                nc.sync.dma_start(
                    out=a2a_in[b][2 * qc:2 * qc + 2].rearrange(
                        "e p c -> p e c"),
                    in_=stg[:].rearrange("p (e c) -> p e c", e=2))

            def emit_a2a(b):
                nc.gpsimd.collective_compute(
                    "AllToAll", mybir.AluOpType.bypass,
                    replica_groups=[list(range(NCORES))],
                    ins=[a2a_in[b].opt()], outs=[a2a_out[b].opt()],
                )

            # ---------- phase C: output projections (as quanta) ----------
            def gen_woproj(b, wo_sb, cm):
                for mt in range(2):
                    ot = outp.tile([128, D_MODEL], F16, tag="out", name="ot")
                    for nn in range(2):
                        po = ps.tile([128, QC], F32, tag="m1", bufs=2,
                                     name="po")
                        for i in range(NCORES):
                            nc.tensor.matmul(
                                po, cm[:, i, mt * KT:(mt + 1) * KT],
                                wo_sb[i][:, nn * QC:(nn + 1) * QC],
                                start=(i == 0), stop=(i == NCORES - 1))
                            if i % 2 == 1:
                                yield 426
                        nc.vector.tensor_copy(ot[:, nn * QC:(nn + 1) * QC], po)
                    nc.sync.dma_start(
                        out=out_d[b, mt * KT:(mt + 1) * KT, :],
                        in_=ot[:])
                yield 0

            # ---------- emission schedule ----------
            for _ in gen_chunk(0, 0):
                pass
            wo_sb = emit_wo_loads()

            blocks = [(b, qc) for b in range(BATCH) for qc in range(NCH)]
            prev_av = None
            dummy_gen = gen_dummy(N_DUMMY)
            for i, blk in enumerate(blocks):
                cg = None
                if i + 1 < len(blocks):
                    cg = gen_chunk(*blocks[i + 1])
                    push_filler(cg)
                if prev_av is not None:
                    push_filler(prev_av)
                if i == 7:
                    push_filler(dummy_gen)
                g = gen_logits(*blk)
                for _ in g:
                    pass
                if cg is not None:
                    flush_until_done(cg)
                if blk != (1, 3):
                    if blk == (0, 3):
                        def av_then_a2a0():
                            for q in gen_av(0, 3):
                                yield q
                            emit_a2a(0)
                        prev_av = av_then_a2a0()
                    else:
                        prev_av = gen_av(*blk)
                else:
                    # drain everything except the dummies, then the last AV
                    filler_list = [g2 for g2 in filler if g2 is not dummy_gen]
                    filler.clear()
                    filler.extend(filler_list)
                    flush_filler()
                    for _ in gen_av(1, 3):
                        pass
            emit_a2a(1)
            cm0 = wp.tile([DIMS, NCORES, 2 * KT], F16, tag="cm0", name="cm0")
            nc.sync.dma_start(
                out=cm0, in_=a2a_out[0][:].rearrange("e p c -> p e c"))
            cm1 = wp.tile([DIMS, NCORES, 2 * KT], F16, tag="cm1", name="cm1")
            nc.sync.dma_start(
                out=cm1, in_=a2a_out[1][:].rearrange("e p c -> p e c"))

            for _ in gen_woproj(0, wo_sb, cm0):
                pass
            # keep PE p-state warm while the last AllToAll is in flight
            for _ in dummy_gen:
                pass
            for _ in gen_woproj(1, wo_sb, cm1):
                pass

    nc.compile()
    return nc


def _host_prep(inputs):
    x = np.asarray(inputs["in_features"], dtype=np.float32)
    tp = np.asarray(inputs["token_positions"], dtype=np.int32)
    wq = np.asarray(inputs["wq"], dtype=np.float32)
    wk = np.asarray(inputs["wk"], dtype=np.float32)
    wv = np.asarray(inputs["wv"], dtype=np.float32)
    wo = np.asarray(inputs["wo"], dtype=np.float32)

    xT = np.ascontiguousarray(
        np.concatenate([x[b].T for b in range(BATCH)], axis=1)).astype(np.float16)
    woT = np.ascontiguousarray(wo.T).astype(np.float16)

    # cos/sin tables, (dim row, batch*token col); sign baked into S so that
    # rot = x*C + swap(x)*S
    half = D_K // 2
    inv_freq = 1.0 / (THETA ** (2.0 * np.arange(half) / D_K))     # (32,)
    ang = tp.astype(np.float64)[:, :, None] * inv_freq[None, None, :]
    cos = np.cos(ang)                                             # (B, S, 32)
    sin = np.sin(ang)
    rows = np.arange(DIMS)
    j = (rows % D_K) // 2                                         # freq index
    sign = np.where(rows % 2 == 0, -1.0, 1.0)
    csC = np.empty((DIMS, BATCH * SEQ), dtype=np.float16)
    csS = np.empty((DIMS, BATCH * SEQ), dtype=np.float16)
    for b in range(BATCH):
        csC[:, b * SEQ:(b + 1) * SEQ] = cos[b][:, j].T
        csS[:, b * SEQ:(b + 1) * SEQ] = (sin[b][:, j] * sign[None, :]).T

    psw = np.zeros((128, 128), dtype=np.float16)
    r = np.arange(128)
    psw[r ^ 1, r] = 1.0

    in_maps = []
    for c in range(NCORES):
        rsl = slice(DIMS * c, DIMS * (c + 1))
        in_maps.append({
            "xT": xT,
            "psw": psw,
            "wqT": np.ascontiguousarray(wq[rsl].T).astype(np.float16),
            "wkT": np.ascontiguousarray(wk[rsl].T).astype(np.float16),
            "wvT": np.ascontiguousarray(wv[rsl].T).astype(np.float16),
            "woT": woT,
            "csC": csC,
            "csS": csS,
        })
    return in_maps


def kernel(**inputs) -> np.ndarray:
    from concourse.bass_utils import run_bass_kernel_spmd

    if "nc" not in _CACHE:
        _CACHE["nc"] = _build_program()
    nc = _CACHE["nc"]

    in_maps = _host_prep(inputs)
    res = run_bass_kernel_spmd(nc, in_maps, list(range(NCORES))).results

    out = np.empty((BATCH, SEQ, D_MODEL), dtype=np.float32)
    for c in range(NCORES):
        # half-chunk layout for both batches (qc = c//2, half = c%2)
        t0 = (c // 2) * QC + (c % 2) * 2 * KT
        for b in range(BATCH):
            out[b, t0:t0 + 2 * KT, :] = res[c]["out"][b].astype(np.float32)
    return out


# revision 17
# speedup vs baseline: 1.0429x; 1.0057x over previous
"""Causal multi-head self-attention with RoPE on 8 Trainium2 NeuronCores.

Sharding: tensor-parallel over heads — core c owns heads (2c, 2c+1) for BOTH
batch elements.  Feature dim on partitions, tokens on the free dim.

v2: list-scheduled emission.  The PE stream is paced explicitly: the logits
stream (which is throttled by the Activation engine's exp throughput through
the 2-deep logits psum rotation) is interleaved at k-tile granularity with
"filler" matmul quanta — the next chunk's QKV projections, and late in the
kernel the output projections — so the in-order PE sequencer never idles
waiting for exp.  DMAs all ride the SP queue in production order; the three
AllToAlls sit alone on the Pool queue so each dispatches the moment its
staging completes; RoPE multiplies run on GPSIMD (scalar_tensor_tensor) to
unload DVE; softmax reciprocals are batched (one [128,4] strided reciprocal
per head per chunk instead of eight [128,1]s).

  phase A  per 512-token chunk: qT/kT = W @ x^T (f16, K=1024); vT projected
           directly transposed into 130-col k-tiles with a ones column per
           head (denominator comes free out of the AV matmul); RoPE with
           host-precomputed cos/sin (rot = x*C + swap(x)*S, sign folded
           into S).
  phase B  per (batch, q-chunk): logitsT (k-part, q-free) f16 = kT_h^T@qT_h,
           2 heads packed per [128,1024] psum; e = exp(logits/8) -> f16;
           AV transposed: ctx[q-part,65] += e_blk^T @ [v|1]; triangular mask
           on diagonal k-tiles; batched reciprocal + per-q-tile normalize,
           PE-transpose back to (dim, token), stage f16.
  phase C  batch 0: one 8-core AllToAll (512 KB) of half-chunks; batch 1:
           two quarter-chunk AllToAlls (256 KB each).  Local wo^T projection
           per arrival; dummy matmuls keep the PE p-state warm across the
           final collective.  Host reassembles (2, 256, 1024) per core.
"""
import os
import sys
from collections import deque

import numpy as np

for p in ("/opt/trn_rl_repo", "/root/.axon_site/_ro/trn_rl_repo"):
    if os.path.isdir(p) and p not in sys.path:
        sys.path.insert(0, p)

D_MODEL = 1024
NUM_HEADS = 16
D_K = 64
THETA = 10000.0
BATCH = 2
SEQ = 2048
NCORES = 8
H_PER_CORE = 2
DIMS = H_PER_CORE * D_K   # 128 ctx dims owned per core
QC = 512                  # q-chunk
KT = 128                  # k-tile
SCALE = 0.125             # 1/sqrt(d_k)
N_DUMMY = 200

_CACHE = {}


def _build_program():
    import concourse.mybir as mybir
    import concourse.tile as tile
    from concourse import bacc
    from concourse.masks import make_identity, make_upper_triangular

    F32 = mybir.dt.float32
    F16 = mybir.dt.float16
    AFT = mybir.ActivationFunctionType
    ALU = mybir.AluOpType

    nc = bacc.Bacc("TRN2", target_bir_lowering=False, debug=False,
                   num_devices=NCORES)

    xT_d = nc.declare_dram_parameter("xT", [D_MODEL, BATCH * SEQ], F16,
                                     isOutput=False)
    wqT_d = nc.declare_dram_parameter("wqT", [D_MODEL, DIMS], F16, isOutput=False)
    wkT_d = nc.declare_dram_parameter("wkT", [D_MODEL, DIMS], F16, isOutput=False)
    wvT_d = nc.declare_dram_parameter("wvT", [D_MODEL, DIMS], F16, isOutput=False)
    woT_d = nc.declare_dram_parameter("woT", [D_MODEL, D_MODEL], F16, isOutput=False)
    csC_d = nc.declare_dram_parameter("csC", [DIMS, BATCH * SEQ], F16,
                                      isOutput=False)
    csS_d = nc.declare_dram_parameter("csS", [DIMS, BATCH * SEQ], F16,
                                      isOutput=False)
    psw_d = nc.declare_dram_parameter("psw", [128, 128], F16, isOutput=False)
    out_d = nc.declare_dram_parameter("out", [BATCH, 2 * KT, D_MODEL], F16,
                                      isOutput=True)

    NCH = SEQ // QC           # 4 chunks per batch
    NVT = QC // KT            # 4 k-tiles per chunk

    with tile.TileContext(nc) as tc:
        with tc.tile_pool(name="consts", bufs=1) as consts, \
             tc.tile_pool(name="qk", bufs=1) as qkp, \
             tc.tile_pool(name="vbufp", bufs=1) as vbufp, \
             tc.tile_pool(name="ps", bufs=1, space="PSUM") as ps, \
             tc.tile_pool(name="epool", bufs=26) as epool, \
             tc.tile_pool(name="rawp", bufs=2) as rawp, \
             tc.tile_pool(name="xtp", bufs=1) as xtp, \
             tc.tile_pool(name="ropep", bufs=2) as ropep, \
             tc.tile_pool(name="normp", bufs=3) as normp, \
             tc.tile_pool(name="stp", bufs=2) as stp, \
             tc.tile_pool(name="wp", bufs=1) as wp, \
             tc.tile_pool(name="outp", bufs=2) as outp, \
             tc.tile_pool(name="dram", bufs=1, space="DRAM") as dram:

            # ---------- constants ----------
            tri_f = consts.tile([KT, KT], F32)
            make_upper_triangular(nc, tri_f[:], val=1.0, diag=True)
            tri = consts.tile([KT, KT], F16)
            nc.vector.tensor_copy(tri, tri_f)
            ident = consts.tile([128, 128], F32)
            make_identity(nc, ident[:])
            ones16 = consts.tile([128, 16], F16)
            nc.vector.memset(ones16, 1.0)

            csC = consts.tile([DIMS, BATCH * SEQ], F16, name="csC")
            csS = consts.tile([DIMS, BATCH * SEQ], F16, name="csS")
            psw = consts.tile([128, 128], F16, name="psw")

            a2a_in = [dram.tile([NCORES, DIMS, 2 * KT], F16, name=f"a2ain{b}")
                      for b in range(BATCH)]
            a2a_out = [dram.tile([NCORES, DIMS, 2 * KT], F16, name=f"a2aout{b}")
                       for b in range(BATCH)]

            qR = {b: qkp.tile([DIMS, SEQ], F16, tag=f"qR{b}", name=f"qR{b}")
                  for b in range(BATCH)}
            kR = {b: qkp.tile([DIMS, SEQ], F16, tag=f"kR{b}", name=f"kR{b}")
                  for b in range(BATCH)}
            vbuf = {b: vbufp.tile([128, 130 * (SEQ // KT)], F16, tag=f"vb{b}",
                                  name=f"vbuf{b}")
                    for b in range(BATCH)}

            # projection weights (wq first so the very first matmul can start
            # as soon as wq + the first x chunk land)
            w_sb = {}
            for nm, d in (("q", wqT_d), ("k", wkT_d), ("v", wvT_d)):
                wt = wp.tile([128, 8, DIMS], F16, tag=f"w{nm}", name=f"w{nm}")
                w_sb[nm] = [wt[:, k8, :] for k8 in range(8)]
                w_sb[nm + "_t"] = wt

            xt_all = [xtp.tile([128, BATCH * SEQ], F16, tag=f"xt{k8}",
                               name=f"xt{k8}")
                      for k8 in range(8)]

            def load_x(c0, c1):
                for k8 in range(8):
                    nc.sync.dma_start(
                        out=xt_all[k8][:, c0:c1],
                        in_=xT_d[k8 * 128:(k8 + 1) * 128, c0:c1])

            def load_w(nm, d):
                nc.sync.dma_start(
                    out=w_sb[nm + "_t"],
                    in_=d[:].rearrange("(e p) c -> p e c", p=128))

            def load_cs(c0, c1):
                nc.sync.dma_start(out=csC[:, c0:c1], in_=csC_d[:, c0:c1])
                nc.sync.dma_start(out=csS[:, c0:c1], in_=csS_d[:, c0:c1])

            load_w("q", wqT_d)
            for k8 in range(4):
                nc.sync.dma_start(out=xt_all[k8][:, 0:QC],
                                  in_=xT_d[k8 * 128:(k8 + 1) * 128, 0:QC])
            load_w("k", wkT_d)
            for k8 in range(4, 8):
                nc.sync.dma_start(out=xt_all[k8][:, 0:QC],
                                  in_=xT_d[k8 * 128:(k8 + 1) * 128, 0:QC])
            load_cs(0, QC)
            load_w("v", wvT_d)
            nc.sync.dma_start(out=psw, in_=psw_d[:])
            load_x(QC, SEQ)        # rest of batch 0, 8 big DMAs
            load_cs(QC, SEQ)
            load_cs(SEQ, 2 * SEQ)
            load_x(SEQ, 2 * SEQ)   # batch 1, 8 big DMAs

            def emit_wo_loads():
                t = wp.tile([128, 8, D_MODEL], F16, tag="wo", name="wo")
                nc.sync.dma_start(
                    out=t, in_=woT_d[:].rearrange("(e p) c -> p e c", p=128))
                return [t[:, k8, :] for k8 in range(8)]

            # ---------- phase A: one 512-token chunk, as filler quanta ----
            # generator yields (approx_pe_ns) after each quantum
            def gen_chunk(b, t):
                g0 = b * SEQ + t * QC
                c0 = t * QC
                rqk = rawp.tile([DIMS, 2 * QC], F16, tag="rawqk", name="rawqk")
                sw = {}
                for ni, nm in enumerate(("q", "k")):
                    pp = ps.tile([128, QC], F32, tag="m1", bufs=2, name="pp")
                    for k8 in range(8):
                        nc.tensor.matmul(pp, w_sb[nm][k8],
                                         xt_all[k8][:, g0:g0 + QC],
                                         start=(k8 == 0), stop=(k8 == 7))
                        if k8 % 2 == 1:
                            yield 426
                    nc.vector.tensor_copy(rqk[:, ni * QC:(ni + 1) * QC], pp)
                    # even/odd row swap for RoPE: PE permutation matmul
                    sw[ni] = ps.tile([128, QC], F32, tag="m1", bufs=2,
                                     name="sw")
                    nc.tensor.matmul(sw[ni], psw,
                                     rqk[:, ni * QC:(ni + 1) * QC],
                                     start=True, stop=True)
                    yield 213
                    nsl = slice(ni * QC, (ni + 1) * QC)
                    t1 = ropep.tile([DIMS, QC], F16, tag=f"t1{ni}",
                                    name="t1")
                    swm = ropep.tile([DIMS, QC], F16, tag=f"sw{ni}",
                                     name="swm")
                    nc.vector.tensor_mul(t1, rqk[:, nsl], csC[:, g0:g0 + QC])
                    nc.vector.tensor_mul(swm, sw[ni], csS[:, g0:g0 + QC])
                    dst = (qR if nm == "q" else kR)[b][:, c0:c0 + QC]
                    nc.vector.tensor_add(dst, t1, swm)

                # v: projected directly transposed, (token, dim) per 128-tile
                pv = ps.tile([128, QC], F32, tag="m1", bufs=2, name="pv")
                for i in range(NVT):
                    for k8 in range(8):
                        nc.tensor.matmul(
                            pv[:, i * KT:(i + 1) * KT],
                            xt_all[k8][:, g0 + i * KT:g0 + (i + 1) * KT],
                            w_sb["v"][k8],
                            start=(k8 == 0), stop=(k8 == 7),
                            skip_group_check=True)
                    yield 426
                vb = vbuf[b]
                cview = vb[:, 130 * NVT * t:130 * NVT * (t + 1)]
                v_view = cview.rearrange("p (n c) -> p n c", c=130)
                for col in (64, 129):
                    nc.vector.tensor_copy(
                        v_view[:, :, col:col + 1].rearrange("p n c -> p (n c)"),
                        ones16[:, 0:NVT])
                dst = cview.rearrange("p (n h c) -> p n h c", h=2, c=65)[
                    :, :, :, 0:64]
                src = pv[:].rearrange("p (n h c) -> p n h c", h=2, c=64)
                nc.vector.tensor_copy(dst, src)
                yield 0

            def gen_dummy(n):
                for _ in range(n):
                    dpl = ps.tile([128, QC], F32, tag="m1", bufs=2,
                                  name="dpl")
                    nc.tensor.matmul(dpl[0:16, 0:256], ones16,
                                     xt_all[0][:, 0:256],
                                     start=True, stop=True)
                    yield 107

            # ---------- filler management ----------
            filler = deque()   # (pe_ns, generator)

            def push_filler(gen):
                filler.append(gen)

            def pop_filler(budget_ns):
                spent = 0
                while filler and spent < budget_ns:
                    g = filler[0]
                    try:
                        spent += next(g)
                    except StopIteration:
                        filler.popleft()
                return spent

            def flush_filler():
                while filler:
                    g = filler[0]
                    try:
                        next(g)
                    except StopIteration:
                        filler.popleft()

            def flush_until_done(cg):
                while any(g is cg for g in filler):
                    g = filler[0]
                    try:
                        next(g)
                    except StopIteration:
                        filler.popleft()

            # ---------- phase B ----------
            ets_store = {}

            def gen_logits(b, qc):
                nkt = NVT * qc + NVT
                ets = []
                ets_store[(b, qc)] = ets
                for kt in range(nkt):
                    j = kt - NVT * qc
                    q0 = 0 if j < 0 else KT * j
                    pl = ps.tile([128, 2 * QC], F32, tag="logit", bufs=2,
                                 name="pl")
                    for h in range(H_PER_CORE):
                        nc.tensor.matmul(
                            pl[:, h * QC + q0:(h + 1) * QC],
                            kR[b][64 * h:64 * (h + 1), kt * KT:(kt + 1) * KT],
                            qR[b][64 * h:64 * (h + 1),
                                  qc * QC + q0:(qc + 1) * QC],
                            start=True, stop=True)
                    et = epool.tile([128, 2 * QC], F16, tag="e", name="et")
                    if q0 == 0:
                        nc.scalar.activation(et, pl, AFT.Exp, scale=SCALE)
                    else:
                        ev = et[:].rearrange("p (h n) -> p h n", h=2)[
                            :, :, q0:QC]
                        pv_ = pl[:].rearrange("p (h n) -> p h n", h=2)[
                            :, :, q0:QC]
                        nc.scalar.activation(ev, pv_, AFT.Exp, scale=SCALE)
                    if j >= 0:
                        for h in range(H_PER_CORE):
                            msl = slice(h * QC + q0, h * QC + q0 + KT)
                            nc.vector.tensor_mul(et[:, msl], et[:, msl], tri)
                    ets.append(et)
                    # exp takes ~975ns/kt on Act; the 2 logits mms are
                    # ~426ns: pad the difference with filler
                    pop_filler(550)
                    yield

            def gen_av(b, qc):
                ets = ets_store.pop((b, qc))
                pctx = [ps.tile([128, QC], F32, tag=f"ctx{h}", bufs=1,
                                name=f"pctx{h}")
                        for h in range(H_PER_CORE)]
                for qt in range(NVT):
                    cnt = 0
                    for kt in range(NVT * qc + qt + 1):
                        for h in range(H_PER_CORE):
                            vt = vbuf[b][:, 130 * kt + 65 * h:
                                         130 * kt + 65 * h + 65]
                            nc.tensor.matmul(
                                pctx[h][:, qt * KT:qt * KT + 65],
                                ets[kt][:, h * QC + qt * KT:
                                        h * QC + (qt + 1) * KT],
                                vt,
                                start=(kt == 0),
                                stop=(kt == NVT * qc + qt),
                                skip_group_check=True)
                        cnt += 1
                        if cnt % 6 == 0:
                            yield 324
                    yield 110

                # epilogue: batched reciprocal, normalize, transpose, stage
                rc = {}
                for h in range(H_PER_CORE):
                    rc[h] = normp.tile([128, NVT], F32, tag=f"rc{h}",
                                       name="rc")
                    den = pctx[h][:].rearrange("p (n c) -> p n c", c=KT)[
                        :, :, 64:65].rearrange("p n c -> p (n c)")
                    nc.vector.reciprocal(rc[h], den)
                stg = stp.tile([128, QC], F16, tag="stage", name="stage")
                for qt in range(NVT):
                    tp = ps.tile([128, QC], F32, tag="m1", bufs=2, name="tp")
                    nr = normp.tile([128, 128], F32, tag="nr", name="nr")
                    for h in range(H_PER_CORE):
                        nc.vector.tensor_scalar_mul(
                            nr[:, 64 * h:64 * (h + 1)],
                            pctx[h][:, qt * KT:qt * KT + 64],
                            rc[h][:, qt:qt + 1])
                    nc.tensor.transpose(tp[:, 0:128], nr, ident[:])
                    nc.vector.tensor_copy(stg[:, qt * KT:(qt + 1) * KT],
                                          tp[:, 0:128])
                nc.sync.dma_start(
                    out=a2a_in[b][2 * qc:2 * qc + 2].rearrange(
                        "e p c -> p e c"),
                    in_=stg[:].rearrange("p (e c) -> p e c", e=2))

            def emit_a2a(b):
                nc.gpsimd.collective_compute(
                    "AllToAll", mybir.AluOpType.bypass,
                    replica_groups=[list(range(NCORES))],
                    ins=[a2a_in[b].opt()], outs=[a2a_out[b].opt()],
                )

            # ---------- phase C: output projections (as quanta) ----------
            def gen_woproj(b, wo_sb, cm):
                for mt in range(2):
                    ot = outp.tile([128, D_MODEL], F16, tag="out", name="ot")
                    for nn in range(2):
                        po = ps.tile([128, QC], F32, tag="m1", bufs=2,
                                     name="po")
                        for i in range(NCORES):
                            nc.tensor.matmul(
                                po, cm[:, i, mt * KT:(mt + 1) * KT],
                                wo_sb[i][:, nn * QC:(nn + 1) * QC],
                                start=(i == 0), stop=(i == NCORES - 1))
                            if i % 2 == 1:
                                yield 426
                        nc.vector.tensor_copy(ot[:, nn * QC:(nn + 1) * QC], po)
                    nc.sync.dma_start(
                        out=out_d[b, mt * KT:(mt + 1) * KT, :],
                        in_=ot[:])
                yield 0

            # ---------- emission schedule ----------
            for _ in gen_chunk(0, 0):
                pass
            wo_sb = emit_wo_loads()

            blocks = [(b, qc) for b in range(BATCH) for qc in range(NCH)]
            prev_av = None
            dummy_gen = gen_dummy(N_DUMMY)
            for i, blk in enumerate(blocks):
                cg = None
                if i + 1 < len(blocks):
                    cg = gen_chunk(*blocks[i + 1])
                    push_filler(cg)
                if prev_av is not None:
                    push_filler(prev_av)
                if i == 7:
                    push_filler(dummy_gen)
                g = gen_logits(*blk)
                for _ in g:
                    pass
                if cg is not None:
                    flush_until_done(cg)
                if blk != (1, 3):
                    if blk == (0, 3):
                        def av_then_a2a0():
                            for q in gen_av(0, 3):
                                yield q
                            emit_a2a(0)
                        prev_av = av_then_a2a0()
                    else:
                        prev_av = gen_av(*blk)
                else:
                    # drain everything except the dummies, then the last AV
                    filler_list = [g2 for g2 in filler if g2 is not dummy_gen]
                    filler.clear()
                    filler.extend(filler_list)
                    flush_filler()
                    for _ in gen_av(1, 3):
                        pass
            emit_a2a(1)
            cm0 = wp.tile([DIMS, NCORES, 2 * KT], F16, tag="cm0", name="cm0")
            nc.sync.dma_start(
                out=cm0, in_=a2a_out[0][:].rearrange("e p c -> p e c"))
            cm1 = wp.tile([DIMS, NCORES, 2 * KT], F16, tag="cm1", name="cm1")
            nc.sync.dma_start(
                out=cm1, in_=a2a_out[1][:].rearrange("e p c -> p e c"))

            for _ in gen_woproj(0, wo_sb, cm0):
                pass
            # keep PE p-state warm while the last AllToAll is in flight
            for _ in dummy_gen:
                pass
            for _ in gen_woproj(1, wo_sb, cm1):
                pass

    nc.compile()
    return nc


def _host_prep(inputs):
    x = np.asarray(inputs["in_features"], dtype=np.float32)
    tp = np.asarray(inputs["token_positions"], dtype=np.int32)
    wq = np.asarray(inputs["wq"], dtype=np.float32)
    wk = np.asarray(inputs["wk"], dtype=np.float32)
    wv = np.asarray(inputs["wv"], dtype=np.float32)
    wo = np.asarray(inputs["wo"], dtype=np.float32)

    xT = np.ascontiguousarray(
        np.concatenate([x[b].T for b in range(BATCH)], axis=1)).astype(np.float16)
    woT = np.ascontiguousarray(wo.T).astype(np.float16)

    # cos/sin tables, (dim row, batch*token col); sign baked into S so that
    # rot = x*C + swap(x)*S
    half = D_K // 2
    inv_freq = 1.0 / (THETA ** (2.0 * np.arange(half) / D_K))     # (32,)
    ang = tp.astype(np.float64)[:, :, None] * inv_freq[None, None, :]
    cos = np.cos(ang)                                             # (B, S, 32)
    sin = np.sin(ang)
    rows = np.arange(DIMS)
    j = (rows % D_K) // 2                                         # freq index
    sign = np.where(rows % 2 == 0, -1.0, 1.0)
    csC = np.empty((DIMS, BATCH * SEQ), dtype=np.float16)
    csS = np.empty((DIMS, BATCH * SEQ), dtype=np.float16)
    for b in range(BATCH):
        csC[:, b * SEQ:(b + 1) * SEQ] = cos[b][:, j].T
        csS[:, b * SEQ:(b + 1) * SEQ] = (sin[b][:, j] * sign[None, :]).T

    psw = np.zeros((128, 128), dtype=np.float16)
    r = np.arange(128)
    psw[r ^ 1, r] = 1.0

    in_maps = []
    for c in range(NCORES):
        rsl = slice(DIMS * c, DIMS * (c + 1))
        in_maps.append({
            "xT": xT,
            "psw": psw,
            "wqT": np.ascontiguousarray(wq[rsl].T).astype(np.float16),
            "wkT": np.ascontiguousarray(wk[rsl].T).astype(np.float16),
            "wvT": np.ascontiguousarray(wv[rsl].T).astype(np.float16),
            "woT": woT,
            "csC": csC,
            "csS": csS,
        })
    return in_maps


def kernel(**inputs) -> np.ndarray:
    from concourse.bass_utils import run_bass_kernel_spmd

    if "nc" not in _CACHE:
        _CACHE["nc"] = _build_program()
    nc = _CACHE["nc"]

    in_maps = _host_prep(inputs)
    res = run_bass_kernel_spmd(nc, in_maps, list(range(NCORES))).results

    out = np.empty((BATCH, SEQ, D_MODEL), dtype=np.float32)
    for c in range(NCORES):
        # half-chunk layout for both batches (qc = c//2, half = c%2)
        t0 = (c // 2) * QC + (c % 2) * 2 * KT
        for b in range(BATCH):
            out[b, t0:t0 + 2 * KT, :] = res[c]["out"][b].astype(np.float32)
    return out


# revision 18
# speedup vs baseline: 1.0897x; 1.0449x over previous
"""Causal multi-head self-attention with RoPE on 8 Trainium2 NeuronCores.

Sharding: tensor-parallel over heads — core c owns heads (2c, 2c+1) for BOTH
batch elements.  Feature dim on partitions, tokens on the free dim.

v2: list-scheduled emission.  The PE stream is paced explicitly: the logits
stream (which is throttled by the Activation engine's exp throughput through
the 2-deep logits psum rotation) is interleaved at k-tile granularity with
"filler" matmul quanta — the next chunk's QKV projections, and late in the
kernel the output projections — so the in-order PE sequencer never idles
waiting for exp.  DMAs all ride the SP queue in production order; the three
AllToAlls sit alone on the Pool queue so each dispatches the moment its
staging completes; RoPE multiplies run on GPSIMD (scalar_tensor_tensor) to
unload DVE; softmax reciprocals are batched (one [128,4] strided reciprocal
per head per chunk instead of eight [128,1]s).

  phase A  per 512-token chunk: qT/kT = W @ x^T (f16, K=1024); vT projected
           directly transposed into 130-col k-tiles with a ones column per
           head (denominator comes free out of the AV matmul); RoPE with
           host-precomputed cos/sin (rot = x*C + swap(x)*S, sign folded
           into S).
  phase B  per (batch, q-chunk): logitsT (k-part, q-free) f16 = kT_h^T@qT_h,
           2 heads packed per [128,1024] psum; e = exp(logits/8) -> f16;
           AV transposed: ctx[q-part,65] += e_blk^T @ [v|1]; triangular mask
           on diagonal k-tiles; batched reciprocal + per-q-tile normalize,
           PE-transpose back to (dim, token), stage f16.
  phase C  batch 0: one 8-core AllToAll (512 KB) of half-chunks; batch 1:
           two quarter-chunk AllToAlls (256 KB each).  Local wo^T projection
           per arrival; dummy matmuls keep the PE p-state warm across the
           final collective.  Host reassembles (2, 256, 1024) per core.
"""
import os
import sys
from collections import deque

import numpy as np

for p in ("/opt/trn_rl_repo", "/root/.axon_site/_ro/trn_rl_repo"):
    if os.path.isdir(p) and p not in sys.path:
        sys.path.insert(0, p)

D_MODEL = 1024
NUM_HEADS = 16
D_K = 64
THETA = 10000.0
BATCH = 2
SEQ = 2048
NCORES = 8
H_PER_CORE = 2
DIMS = H_PER_CORE * D_K   # 128 ctx dims owned per core
QC = 512                  # q-chunk
KT = 128                  # k-tile
SCALE = 0.125             # 1/sqrt(d_k)
N_DUMMY = 310

_CACHE = {}


def _build_program():
    import concourse.mybir as mybir
    import concourse.tile as tile
    from concourse import bacc
    from concourse.masks import make_identity, make_upper_triangular

    F32 = mybir.dt.float32
    F16 = mybir.dt.float16
    AFT = mybir.ActivationFunctionType
    ALU = mybir.AluOpType

    nc = bacc.Bacc("TRN2", target_bir_lowering=False, debug=False,
                   num_devices=NCORES)

    xT_d = nc.declare_dram_parameter("xT", [D_MODEL, BATCH * SEQ], F16,
                                     isOutput=False)
    wqT_d = nc.declare_dram_parameter("wqT", [D_MODEL, DIMS], F16, isOutput=False)
    wkT_d = nc.declare_dram_parameter("wkT", [D_MODEL, DIMS], F16, isOutput=False)
    wvT_d = nc.declare_dram_parameter("wvT", [D_MODEL, DIMS], F16, isOutput=False)
    woT_d = nc.declare_dram_parameter("woT", [D_MODEL, D_MODEL], F16, isOutput=False)
    csC_d = nc.declare_dram_parameter("csC", [DIMS, BATCH * SEQ], F16,
                                      isOutput=False)
    csS_d = nc.declare_dram_parameter("csS", [DIMS, BATCH * SEQ], F16,
                                      isOutput=False)
    psw_d = nc.declare_dram_parameter("psw", [128, 128], F16, isOutput=False)
    out_d = nc.declare_dram_parameter("out", [BATCH, 2 * KT, D_MODEL], F16,
                                      isOutput=True)

    NCH = SEQ // QC           # 4 chunks per batch
    NVT = QC // KT            # 4 k-tiles per chunk

    with tile.TileContext(nc) as tc:
        with tc.tile_pool(name="consts", bufs=1) as consts, \
             tc.tile_pool(name="qk", bufs=1) as qkp, \
             tc.tile_pool(name="vbufp", bufs=1) as vbufp, \
             tc.tile_pool(name="ps", bufs=1, space="PSUM") as ps, \
             tc.tile_pool(name="epool", bufs=26) as epool, \
             tc.tile_pool(name="rawp", bufs=2) as rawp, \
             tc.tile_pool(name="xtp", bufs=1) as xtp, \
             tc.tile_pool(name="ropep", bufs=2) as ropep, \
             tc.tile_pool(name="normp", bufs=3) as normp, \
             tc.tile_pool(name="stp", bufs=2) as stp, \
             tc.tile_pool(name="wp", bufs=1) as wp, \
             tc.tile_pool(name="outp", bufs=2) as outp, \
             tc.tile_pool(name="dram", bufs=1, space="DRAM") as dram:

            # ---------- constants ----------
            tri_f = consts.tile([KT, KT], F32)
            make_upper_triangular(nc, tri_f[:], val=1.0, diag=True)
            tri = consts.tile([KT, KT], F16)
            nc.vector.tensor_copy(tri, tri_f)
            ident = consts.tile([128, 128], F32)
            make_identity(nc, ident[:])
            ones16 = consts.tile([128, 16], F16)
            nc.vector.memset(ones16, 1.0)

            csC = consts.tile([DIMS, BATCH * SEQ], F16, name="csC")
            csS = consts.tile([DIMS, BATCH * SEQ], F16, name="csS")
            psw = consts.tile([128, 128], F16, name="psw")

            a2a_in = [dram.tile([NCORES, DIMS, 2 * KT], F16, name=f"a2ain{b}")
                      for b in range(BATCH)]
            a2a_out = [dram.tile([NCORES, DIMS, 2 * KT], F16, name=f"a2aout{b}")
                       for b in range(BATCH)]

            qR = {b: qkp.tile([DIMS, SEQ], F16, tag=f"qR{b}", name=f"qR{b}")
                  for b in range(BATCH)}
            kR = {b: qkp.tile([DIMS, SEQ], F16, tag=f"kR{b}", name=f"kR{b}")
                  for b in range(BATCH)}
            vbuf = {b: vbufp.tile([128, 130 * (SEQ // KT)], F16, tag=f"vb{b}",
                                  name=f"vbuf{b}")
                    for b in range(BATCH)}

            # projection weights (wq first so the very first matmul can start
            # as soon as wq + the first x chunk land)
            w_sb = {}
            for nm, d in (("q", wqT_d), ("k", wkT_d), ("v", wvT_d)):
                wt = wp.tile([128, 8, DIMS], F16, tag=f"w{nm}", name=f"w{nm}")
                w_sb[nm] = [wt[:, k8, :] for k8 in range(8)]
                w_sb[nm + "_t"] = wt

            xt_all = [xtp.tile([128, BATCH * SEQ], F16, tag=f"xt{k8}",
                               name=f"xt{k8}")
                      for k8 in range(8)]

            def load_x(c0, c1):
                for k8 in range(8):
                    nc.sync.dma_start(
                        out=xt_all[k8][:, c0:c1],
                        in_=xT_d[k8 * 128:(k8 + 1) * 128, c0:c1])

            def load_w(nm, d):
                nc.sync.dma_start(
                    out=w_sb[nm + "_t"],
                    in_=d[:].rearrange("(e p) c -> p e c", p=128))

            def load_cs(c0, c1):
                nc.sync.dma_start(out=csC[:, c0:c1], in_=csC_d[:, c0:c1])
                nc.sync.dma_start(out=csS[:, c0:c1], in_=csS_d[:, c0:c1])

            load_w("q", wqT_d)
            for k8 in range(4):
                nc.sync.dma_start(out=xt_all[k8][:, 0:QC],
                                  in_=xT_d[k8 * 128:(k8 + 1) * 128, 0:QC])
            load_w("k", wkT_d)
            for k8 in range(4, 8):
                nc.sync.dma_start(out=xt_all[k8][:, 0:QC],
                                  in_=xT_d[k8 * 128:(k8 + 1) * 128, 0:QC])
            load_cs(0, QC)
            load_w("v", wvT_d)
            nc.sync.dma_start(out=psw, in_=psw_d[:])
            load_x(QC, SEQ)        # rest of batch 0, 8 big DMAs
            load_cs(QC, SEQ)
            load_cs(SEQ, 2 * SEQ)
            load_x(SEQ, 2 * SEQ)   # batch 1, 8 big DMAs

            def emit_wo_loads():
                t = wp.tile([128, 8, D_MODEL], F16, tag="wo", name="wo")
                nc.sync.dma_start(
                    out=t, in_=woT_d[:].rearrange("(e p) c -> p e c", p=128))
                return [t[:, k8, :] for k8 in range(8)]

            # ---------- phase A: one 512-token chunk, as filler quanta ----
            # generator yields (approx_pe_ns) after each quantum
            def gen_chunk(b, t):
                g0 = b * SEQ + t * QC
                c0 = t * QC
                rqk = rawp.tile([DIMS, 2 * QC], F16, tag="rawqk", name="rawqk")
                sw = {}
                for ni, nm in enumerate(("q", "k")):
                    pp = ps.tile([128, QC], F32, tag="m1", bufs=2, name="pp")
                    for k8 in range(8):
                        nc.tensor.matmul(pp, w_sb[nm][k8],
                                         xt_all[k8][:, g0:g0 + QC],
                                         start=(k8 == 0), stop=(k8 == 7))
                        if k8 % 2 == 1:
                            yield 426
                    nc.vector.tensor_copy(rqk[:, ni * QC:(ni + 1) * QC], pp)
                    # even/odd row swap for RoPE: PE permutation matmul
                    sw[ni] = ps.tile([128, QC], F32, tag="m1", bufs=2,
                                     name="sw")
                    nc.tensor.matmul(sw[ni], psw,
                                     rqk[:, ni * QC:(ni + 1) * QC],
                                     start=True, stop=True)
                    yield 213
                    nsl = slice(ni * QC, (ni + 1) * QC)
                    t1 = ropep.tile([DIMS, QC], F16, tag=f"t1{ni}",
                                    name="t1")
                    swm = ropep.tile([DIMS, QC], F16, tag=f"sw{ni}",
                                     name="swm")
                    nc.vector.tensor_mul(t1, rqk[:, nsl], csC[:, g0:g0 + QC])
                    nc.vector.tensor_mul(swm, sw[ni], csS[:, g0:g0 + QC])
                    dst = (qR if nm == "q" else kR)[b][:, c0:c0 + QC]
                    nc.vector.tensor_add(dst, t1, swm)

                # v: projected directly transposed, (token, dim) per 128-tile
                pv = ps.tile([128, QC], F32, tag="m1", bufs=2, name="pv")
                for i in range(NVT):
                    for k8 in range(8):
                        nc.tensor.matmul(
                            pv[:, i * KT:(i + 1) * KT],
                            xt_all[k8][:, g0 + i * KT:g0 + (i + 1) * KT],
                            w_sb["v"][k8],
                            start=(k8 == 0), stop=(k8 == 7),
                            skip_group_check=True)
                    yield 426
                vb = vbuf[b]
                cview = vb[:, 130 * NVT * t:130 * NVT * (t + 1)]
                v_view = cview.rearrange("p (n c) -> p n c", c=130)
                for col in (64, 129):
                    nc.vector.tensor_copy(
                        v_view[:, :, col:col + 1].rearrange("p n c -> p (n c)"),
                        ones16[:, 0:NVT])
                dst = cview.rearrange("p (n h c) -> p n h c", h=2, c=65)[
                    :, :, :, 0:64]
                src = pv[:].rearrange("p (n h c) -> p n h c", h=2, c=64)
                nc.vector.tensor_copy(dst, src)
                yield 0

            def gen_dummy(n):
                for _ in range(n):
                    dpl = ps.tile([128, QC], F32, tag="m1", bufs=2,
                                  name="dpl")
                    nc.tensor.matmul(dpl[0:16, 0:256], ones16,
                                     xt_all[0][:, 0:256],
                                     start=True, stop=True)
                    yield 107

            # ---------- filler management ----------
            filler = deque()   # (pe_ns, generator)

            def push_filler(gen):
                filler.append(gen)

            def pop_filler(budget_ns):
                spent = 0
                while filler and spent < budget_ns:
                    g = filler[0]
                    try:
                        spent += next(g)
                    except StopIteration:
                        filler.popleft()
                return spent

            def flush_filler():
                while filler:
                    g = filler[0]
                    try:
                        next(g)
                    except StopIteration:
                        filler.popleft()

            def flush_until_done(cg):
                while any(g is cg for g in filler):
                    g = filler[0]
                    try:
                        next(g)
                    except StopIteration:
                        filler.popleft()

            # ---------- phase B ----------
            ets_store = {}

            def gen_logits(b, qc):
                nkt = NVT * qc + NVT
                ets = []
                ets_store[(b, qc)] = ets
                for kt in range(nkt):
                    j = kt - NVT * qc
                    q0 = 0 if j < 0 else KT * j
                    pl = ps.tile([128, 2 * QC], F32, tag="logit", bufs=2,
                                 name="pl")
                    for h in range(H_PER_CORE):
                        nc.tensor.matmul(
                            pl[:, h * QC + q0:(h + 1) * QC],
                            kR[b][64 * h:64 * (h + 1), kt * KT:(kt + 1) * KT],
                            qR[b][64 * h:64 * (h + 1),
                                  qc * QC + q0:(qc + 1) * QC],
                            start=True, stop=True)
                    et = epool.tile([128, 2 * QC], F16, tag="e", name="et")
                    if q0 == 0:
                        nc.scalar.activation(et, pl, AFT.Exp, scale=SCALE)
                    else:
                        ev = et[:].rearrange("p (h n) -> p h n", h=2)[
                            :, :, q0:QC]
                        pv_ = pl[:].rearrange("p (h n) -> p h n", h=2)[
                            :, :, q0:QC]
                        nc.scalar.activation(ev, pv_, AFT.Exp, scale=SCALE)
                    if j >= 0:
                        for h in range(H_PER_CORE):
                            msl = slice(h * QC + q0, h * QC + q0 + KT)
                            nc.vector.tensor_mul(et[:, msl], et[:, msl], tri)
                    ets.append(et)
                    # exp takes ~975ns/kt on Act; the 2 logits mms are
                    # ~426ns: pad the difference with filler.  Small blocks
                    # absorb proportionally more so the next chunk's RoPE
                    # lands before its logits are needed.
                    pop_filler(2200 // nkt * 4)
                    yield

            def gen_av(b, qc):
                ets = ets_store.pop((b, qc))
                pctx = [ps.tile([128, QC], F32, tag=f"ctx{h}", bufs=1,
                                name=f"pctx{h}")
                        for h in range(H_PER_CORE)]
                for qt in range(NVT):
                    cnt = 0
                    for kt in range(NVT * qc + qt + 1):
                        for h in range(H_PER_CORE):
                            vt = vbuf[b][:, 130 * kt + 65 * h:
                                         130 * kt + 65 * h + 65]
                            nc.tensor.matmul(
                                pctx[h][:, qt * KT:qt * KT + 65],
                                ets[kt][:, h * QC + qt * KT:
                                        h * QC + (qt + 1) * KT],
                                vt,
                                start=(kt == 0),
                                stop=(kt == NVT * qc + qt),
                                skip_group_check=True)
                        cnt += 1
                        if cnt % 6 == 0:
                            yield 324
                    yield 110

                # epilogue: batched reciprocal, normalize, transpose, stage
                rc = {}
                for h in range(H_PER_CORE):
                    rc[h] = normp.tile([128, NVT], F32, tag=f"rc{h}",
                                       name="rc")
                    den = pctx[h][:].rearrange("p (n c) -> p n c", c=KT)[
                        :, :, 64:65].rearrange("p n c -> p (n c)")
                    nc.vector.reciprocal(rc[h], den)
                stg = stp.tile([128, QC], F16, tag="stage", name="stage")
                for qt in range(NVT):
                    tp = ps.tile([128, QC], F32, tag="m1", bufs=2, name="tp")
                    nr = normp.tile([128, 128], F32, tag="nr", name="nr")
                    for h in range(H_PER_CORE):
                        nc.vector.tensor_scalar_mul(
                            nr[:, 64 * h:64 * (h + 1)],
                            pctx[h][:, qt * KT:qt * KT + 64],
                            rc[h][:, qt:qt + 1])
                    nc.tensor.transpose(tp[:, 0:128], nr, ident[:])
                    nc.vector.tensor_copy(stg[:, qt * KT:(qt + 1) * KT],
                                          tp[:, 0:128])
                nc.sync.dma_start(
                    out=a2a_in[b][2 * qc:2 * qc + 2].rearrange(
                        "e p c -> p e c"),
                    in_=stg[:].rearrange("p (e c) -> p e c", e=2))

            def emit_a2a(b):
                nc.gpsimd.collective_compute(
                    "AllToAll", mybir.AluOpType.bypass,
                    replica_groups=[list(range(NCORES))],
                    ins=[a2a_in[b].opt()], outs=[a2a_out[b].opt()],
                )

            # ---------- phase C: output projections (as quanta) ----------
            def gen_woproj(b, wo_sb, cm):
                for mt in range(2):
                    ot = outp.tile([128, D_MODEL], F16, tag="out", name="ot")
                    for nn in range(2):
                        po = ps.tile([128, QC], F32, tag="m1", bufs=2,
                                     name="po")
                        for i in range(NCORES):
                            nc.tensor.matmul(
                                po, cm[:, i, mt * KT:(mt + 1) * KT],
                                wo_sb[i][:, nn * QC:(nn + 1) * QC],
                                start=(i == 0), stop=(i == NCORES - 1))
                            if i % 2 == 1:
                                yield 426
                        nc.vector.tensor_copy(ot[:, nn * QC:(nn + 1) * QC], po)
                    nc.sync.dma_start(
                        out=out_d[b, mt * KT:(mt + 1) * KT, :],
                        in_=ot[:])
                yield 0

            # ---------- emission schedule ----------
            for _ in gen_chunk(0, 0):
                pass
            wo_sb = emit_wo_loads()

            blocks = [(b, qc) for b in range(BATCH) for qc in range(NCH)]
            cgs = {}
            prev_av = None
            dummy_gen = gen_dummy(N_DUMMY)
            for i, blk in enumerate(blocks):
                # push chunk generators two blocks ahead of their deadline
                for j in ([1, 2] if i == 0 else [2]):
                    if i + j < len(blocks) and blocks[i + j] not in cgs:
                        cgs[blocks[i + j]] = gen_chunk(*blocks[i + j])
                        push_filler(cgs[blocks[i + j]])
                if prev_av is not None:
                    push_filler(prev_av)
                if i == 7:
                    push_filler(dummy_gen)
                g = gen_logits(*blk)
                for _ in g:
                    pass
                if i + 1 < len(blocks):
                    flush_until_done(cgs[blocks[i + 1]])
                if blk != (1, 3):
                    if blk == (0, 3):
                        def av_then_a2a0():
                            for q in gen_av(0, 3):
                                yield q
                            emit_a2a(0)
                        prev_av = av_then_a2a0()
                    else:
                        prev_av = gen_av(*blk)
                else:
                    # drain everything except the dummies, then the last AV
                    filler_list = [g2 for g2 in filler if g2 is not dummy_gen]
                    filler.clear()
                    filler.extend(filler_list)
                    flush_filler()
                    for _ in gen_av(1, 3):
                        pass
            emit_a2a(1)
            cm0 = wp.tile([DIMS, NCORES, 2 * KT], F16, tag="cm0", name="cm0")
            nc.sync.dma_start(
                out=cm0, in_=a2a_out[0][:].rearrange("e p c -> p e c"))
            cm1 = wp.tile([DIMS, NCORES, 2 * KT], F16, tag="cm1", name="cm1")
            nc.sync.dma_start(
                out=cm1, in_=a2a_out[1][:].rearrange("e p c -> p e c"))

            for _ in gen_woproj(0, wo_sb, cm0):
                pass
            # keep PE p-state warm while the last AllToAll is in flight
            for _ in dummy_gen:
                pass
            for _ in gen_woproj(1, wo_sb, cm1):
                pass

    nc.compile()
    return nc


def _host_prep(inputs):
    x = np.asarray(inputs["in_features"], dtype=np.float32)
    tp = np.asarray(inputs["token_positions"], dtype=np.int32)
    wq = np.asarray(inputs["wq"], dtype=np.float32)
    wk = np.asarray(inputs["wk"], dtype=np.float32)
    wv = np.asarray(inputs["wv"], dtype=np.float32)
    wo = np.asarray(inputs["wo"], dtype=np.float32)

    xT = np.ascontiguousarray(
        np.concatenate([x[b].T for b in range(BATCH)], axis=1)).astype(np.float16)
    woT = np.ascontiguousarray(wo.T).astype(np.float16)

    # cos/sin tables, (dim row, batch*token col); sign baked into S so that
    # rot = x*C + swap(x)*S
    half = D_K // 2
    inv_freq = 1.0 / (THETA ** (2.0 * np.arange(half) / D_K))     # (32,)
    ang = tp.astype(np.float64)[:, :, None] * inv_freq[None, None, :]
    cos = np.cos(ang)                                             # (B, S, 32)
    sin = np.sin(ang)
    rows = np.arange(DIMS)
    j = (rows % D_K) // 2                                         # freq index
    sign = np.where(rows % 2 == 0, -1.0, 1.0)
    csC = np.empty((DIMS, BATCH * SEQ), dtype=np.float16)
    csS = np.empty((DIMS, BATCH * SEQ), dtype=np.float16)
    for b in range(BATCH):
        csC[:, b * SEQ:(b + 1) * SEQ] = cos[b][:, j].T
        csS[:, b * SEQ:(b + 1) * SEQ] = (sin[b][:, j] * sign[None, :]).T

    psw = np.zeros((128, 128), dtype=np.float16)
    r = np.arange(128)
    psw[r ^ 1, r] = 1.0

    in_maps = []
    for c in range(NCORES):
        rsl = slice(DIMS * c, DIMS * (c + 1))
        in_maps.append({
            "xT": xT,
            "psw": psw,
            "wqT": np.ascontiguousarray(wq[rsl].T).astype(np.float16),
            "wkT": np.ascontiguousarray(wk[rsl].T).astype(np.float16),
            "wvT": np.ascontiguousarray(wv[rsl].T).astype(np.float16),
            "woT": woT,
            "csC": csC,
            "csS": csS,
        })
    return in_maps


def kernel(**inputs) -> np.ndarray:
    from concourse.bass_utils import run_bass_kernel_spmd

    if "nc" not in _CACHE:
        _CACHE["nc"] = _build_program()
    nc = _CACHE["nc"]

    in_maps = _host_prep(inputs)
    res = run_bass_kernel_spmd(nc, in_maps, list(range(NCORES))).results

    out = np.empty((BATCH, SEQ, D_MODEL), dtype=np.float32)
    for c in range(NCORES):
        # half-chunk layout for both batches (qc = c//2, half = c%2)
        t0 = (c // 2) * QC + (c % 2) * 2 * KT
        for b in range(BATCH):
            out[b, t0:t0 + 2 * KT, :] = res[c]["out"][b].astype(np.float32)
    return out
